# revision 3
# baseline (speedup 1.0000x reference)
"""Multi-head attention (B=8, S=1024, D=1024, H=16) on 8 TRN2 NeuronCores.

Sharding: pure data-parallel over batch — core b computes batch b entirely
locally (no collectives). All matmuls run in bf16 with fp32 PSUM accumulation.

Per-core dataflow (host pre-transposes inputs/weights so no on-chip input
transposes are needed):
  Q_t[d,s] = (WQ.T/8).T-matmul  (scale 1/sqrt(dk) folded into WQ and bq)
  K_t[d,s], V[s,d] (V stored with a ones-column interleaved per head so the
  attention-value matmul also produces softmax row sums)
  per head: S.T[k,q] = K_t_h.T @ Q_t_h  (single K=64 matmul per chunk)
            E.T = exp(S.T) * mask.T     (ACT exp from PSUM, DVE mask mult)
            psum[q, 0:65] = sum_k E.T_tile.T @ [V_h | 1]  -> out + rowsum
            attnout[q, d_h] = psum[:,0:64] * recip(psum[:,64])
  attnout transposed via PE -> WO projection -> + bias -> out[s,o] fp32
"""

import os
import sys
from contextlib import ExitStack

import numpy as np

if "JAX_PLATFORMS" in os.environ and os.environ["JAX_PLATFORMS"] == "cpu":
    # bass execution needs the neuron/axon jax backend
    del os.environ["JAX_PLATFORMS"]

for _p in ("/opt/trn_rl_repo",):
    if _p not in sys.path and os.path.isdir(_p):
        sys.path.insert(0, _p)

import ml_dtypes

import concourse.bass as bass
import concourse.mybir as mybir
import concourse.tile as tile
from concourse import bacc
from concourse.bass import ds, ts
from concourse.bass_utils import run_bass_kernel_spmd
from concourse.masks import make_identity

BF16 = mybir.dt.bfloat16
F32 = mybir.dt.float32
NPBF = ml_dtypes.bfloat16

B, S, D, H, DK = 8, 1024, 1024, 16, 64
P = 128
NT = D // P  # 8 tiles along any 1024 dim
CH = 512  # matmul moving-dim chunk (one PSUM bank of fp32)
NCH = S // CH  # 2

TRACE = False
LAST_RESULTS = None

_NC_CACHE = None


def build_nc():
    nc = bacc.Bacc("TRN2", target_bir_lowering=False, debug=False)

    xq = nc.dram_tensor("xq", [D, S], BF16, kind="ExternalInput")  # q[b].T
    xk = nc.dram_tensor("xk", [D, S], BF16, kind="ExternalInput")
    xv = nc.dram_tensor("xv", [D, S], BF16, kind="ExternalInput")
    wq = nc.dram_tensor("wq", [D, D], BF16, kind="ExternalInput")  # (WQ_w/8).T
    wk = nc.dram_tensor("wk", [D, D], BF16, kind="ExternalInput")  # WK_w.T
    wv = nc.dram_tensor("wv", [D, D], BF16, kind="ExternalInput")  # WV_w.T
    wo = nc.dram_tensor("wo", [D, D], BF16, kind="ExternalInput")  # WO_w.T
    bq = nc.dram_tensor("bq", [P, NT], F32, kind="ExternalInput")  # WQ_b/8
    bk = nc.dram_tensor("bk", [P, NT], F32, kind="ExternalInput")
    bvb = nc.dram_tensor("bvb", [P, H * 65], BF16, kind="ExternalInput")
    bob = nc.dram_tensor("bob", [P, D], F32, kind="ExternalInput")
    mt = nc.dram_tensor("mt", [S, S], BF16, kind="ExternalInput")  # mask[b,0].T
    out = nc.dram_tensor("out", [S, D], F32, kind="ExternalOutput")

    with tile.TileContext(nc) as tc, ExitStack() as ctx:
        pers = ctx.enter_context(tc.tile_pool(name="pers", bufs=1))
        wld = ctx.enter_context(tc.tile_pool(name="wld", bufs=10))
        xld = ctx.enter_context(tc.tile_pool(name="xld", bufs=10))
        epool = ctx.enter_context(tc.tile_pool(name="epool", bufs=16))
        opool = ctx.enter_context(tc.tile_pool(name="opool", bufs=3))
        rpool = ctx.enter_context(tc.tile_pool(name="rpool", bufs=4))
        ps_st = ctx.enter_context(tc.tile_pool(name="ps_st", bufs=2, space="PSUM"))
        ps_acc = ctx.enter_context(tc.tile_pool(name="ps_acc", bufs=2, space="PSUM"))
        ps_av = ctx.enter_context(tc.tile_pool(name="ps_av", bufs=2, space="PSUM"))

        # ---- persistent tiles ----
        qt = [pers.tile([P, S], BF16, name=f"qt{t}", tag=f"qt{t}") for t in range(NT)]
        kt = [pers.tile([P, S], BF16, name=f"kt{t}", tag=f"kt{t}") for t in range(NT)]
        vv = [
            pers.tile([P, H * 65], BF16, name=f"vv{t}", tag=f"vv{t}")
            for t in range(NT)
        ]
        msk = [pers.tile([P, S], BF16, name=f"mk{t}", tag=f"mk{t}") for t in range(NT)]
        ao = [pers.tile([P, D], BF16, name=f"ao{t}", tag=f"ao{t}") for t in range(NT)]
        aot = [pers.tile([P, S], BF16, name=f"at{t}", tag=f"at{t}") for t in range(NT)]
        ident = pers.tile([P, P], BF16, name="ident", tag="ident")
        bq_sb = pers.tile([P, NT], F32, name="bq_sb", tag="bq_sb")
        bk_sb = pers.tile([P, NT], F32, name="bk_sb", tag="bk_sb")
        bv_sb = pers.tile([P, H * 65], BF16, name="bv_sb", tag="bv_sb")
        bo_sb = pers.tile([P, D], F32, name="bo_sb", tag="bo_sb")

        make_identity(nc, ident)
        nc.sync.dma_start(bq_sb[:], bq[:])
        nc.sync.dma_start(bk_sb[:], bk[:])
        nc.sync.dma_start(bv_sb[:], bvb[:])
        nc.sync.dma_start(bo_sb[:], bob[:])
        for t in range(NT):
            nc.sync.dma_start(msk[t][:], mt[ts(t, P), :])

        # ---- phase 1: projections ----
        # Q_t[o,s] / K_t[o,s]: stationary = W.T tile [i,o], moving = x.T [i,s]
        for name, wdram, xdram, dst, bias in (
            ("q", wq, xq, qt, bq_sb),
            ("k", wk, xk, kt, bk_sb),
        ):
            wsb = []
            xsb = []
            for i in range(NT):
                w_t = wld.tile([P, D], BF16, name=f"w{name}{i}", tag="w")
                nc.sync.dma_start(w_t[:], wdram[ts(i, P), :])
                wsb.append(w_t)
                x_t = xld.tile([P, S], BF16, name=f"x{name}{i}", tag="x")
                nc.sync.dma_start(x_t[:], xdram[ts(i, P), :])
                xsb.append(x_t)
            for ot in range(NT):
                for c in range(NCH):
                    ps = ps_acc.tile([P, CH], F32, name="ps_pj", tag="pj")
                    for i in range(NT):
                        nc.tensor.matmul(
                            ps[:],
                            wsb[i][:, ts(ot, P)],
                            xsb[i][:, ts(c, CH)],
                            start=(i == 0),
                            stop=(i == NT - 1),
                        )
                    nc.vector.tensor_scalar_add(
                        dst[ot][:, ts(c, CH)], ps[:], bias[:, ds(ot, 1)]
                    )

        # V[s, d] with ones-columns: stationary = x.T [i,s], moving = W.T [i,o]
        wsb = []
        xsb = []
        for i in range(NT):
            w_t = wld.tile([P, D], BF16, name=f"wv{i}", tag="w")
            nc.sync.dma_start(w_t[:], wv[ts(i, P), :])
            wsb.append(w_t)
            x_t = xld.tile([P, S], BF16, name=f"xv{i}", tag="x")
            nc.sync.dma_start(x_t[:], xv[ts(i, P), :])
            xsb.append(x_t)
        for st_ in range(NT):
            vt = vv[st_]
            # ones columns (col 64 of each 65-wide head block)
            nc.gpsimd.memset(
                vt.rearrange("p (g c) -> p g c", c=65)[:, :, 64:65], 1.0
            )
            for c in range(NCH):
                ps = ps_acc.tile([P, CH], F32, name="ps_pv", tag="pj")
                for i in range(NT):
                    nc.tensor.matmul(
                        ps[:],
                        xsb[i][:, ts(st_, P)],
                        wsb[i][:, ts(c, CH)],
                        start=(i == 0),
                        stop=(i == NT - 1),
                    )
                # scatter 8 head-blocks of 64 into the 65-strided layout, + bias
                g0 = c * 8
                dst_ap = vt[:, ds(g0 * 65, 8 * 65)].rearrange(
                    "p (g c) -> p g c", c=65
                )[:, :, 0:64]
                bias_ap = bv_sb[:, ds(g0 * 65, 8 * 65)].rearrange(
                    "p (g c) -> p g c", c=65
                )[:, :, 0:64]
                nc.vector.tensor_add(
                    dst_ap, ps.rearrange("p (g c) -> p g c", c=64), bias_ap
                )

        # ---- phase 2: attention per head ----
        for h in range(H):
            t = h // 2
            prow = (h % 2) * 64
            eh = []
            for i in range(NT):
                st_ps = ps_st.tile([P, S], F32, name="st", tag="st")
                for c in range(NCH):
                    nc.tensor.matmul(
                        st_ps[:, ts(c, CH)],
                        kt[t][ds(prow, 64), ts(i, P)],
                        qt[t][ds(prow, 64), ts(c, CH)],
                        start=True,
                        stop=True,
                    )
                e = epool.tile([P, S], BF16, name=f"e{i}", tag="e")
                nc.scalar.activation(e[:], st_ps[:], mybir.ActivationFunctionType.Exp)
                nc.vector.tensor_mul(e[:], e[:], msk[i][:])
                eh.append(e)
            for j in range(NT):
                av = ps_av.tile([P, P], F32, name="av", tag="av")
                for i in range(NT):
                    nc.tensor.matmul(
                        av[:, 0:65],
                        eh[i][:, ts(j, P)],
                        vv[i][:, ds(h * 65, 65)],
                        start=(i == 0),
                        stop=(i == NT - 1),
                    )
                rc = rpool.tile([P, 1], F32, name="rc", tag="rc")
                nc.vector.reciprocal(rc[:], av[:, ds(64, 1)])
                nc.vector.tensor_scalar_mul(
                    ao[j][:, ds(h * 64, 64)], av[:, 0:64], rc[:]
                )
            # transpose finished head-pairs: ao[j][:, t*128:...] -> aot[t]
            if h % 2 == 1:
                for j in range(NT):
                    pt = ps_av.tile([P, P], BF16, name="pt", tag="av")
                    nc.tensor.transpose(pt[:], ao[j][:, ts(t, P)], ident[:])
                    nc.vector.tensor_copy(aot[t][:, ts(j, P)], pt[:])

        # ---- phase 3: output projection ----
        wsb = []
        for i in range(NT):
            w_t = wld.tile([P, D], BF16, name=f"wo{i}", tag="w")
            nc.sync.dma_start(w_t[:], wo[ts(i, P), :])
            wsb.append(w_t)
        for j in range(NT):
            for c in range(NCH):
                ps = ps_acc.tile([P, CH], F32, name="ps_wo", tag="pj")
                for i in range(NT):
                    nc.tensor.matmul(
                        ps[:],
                        aot[i][:, ts(j, P)],
                        wsb[i][:, ts(c, CH)],
                        start=(i == 0),
                        stop=(i == NT - 1),
                    )
                osb = opool.tile([P, CH], F32, name="osb", tag="osb")
                nc.vector.tensor_add(osb[:], ps[:], bo_sb[:, ts(c, CH)])
                nc.sync.dma_start(out[ts(j, P), ts(c, CH)], osb[:])

    nc.compile()
    return nc


def prep_inputs(q, k, v, mask, WQ_w, WQ_b, WK_w, WK_b, WV_w, WV_b, WO_w, WO_b):
    """Build the 8 per-core input maps (host-side layout prep)."""
    f32 = np.float32
    wq_t = np.ascontiguousarray((WQ_w.astype(f32) * 0.125).T).astype(NPBF)
    wk_t = np.ascontiguousarray(WK_w.astype(f32).T).astype(NPBF)
    wv_t = np.ascontiguousarray(WV_w.astype(f32).T).astype(NPBF)
    wo_t = np.ascontiguousarray(WO_w.astype(f32).T).astype(NPBF)
    bq_l = np.ascontiguousarray((WQ_b.astype(f32) * 0.125).reshape(NT, P).T)
    bk_l = np.ascontiguousarray(WK_b.astype(f32).reshape(NT, P).T)
    bvb = np.zeros((P, H * 65), NPBF)
    bv_f = WV_b.astype(f32)
    for h in range(H):
        bvb[:, h * 65 : h * 65 + 64] = bv_f[h * 64 : (h + 1) * 64].astype(NPBF)[None, :]
    bob = np.ascontiguousarray(np.broadcast_to(WO_b.astype(f32), (P, D)))

    in_maps = []
    for b in range(B):
        in_maps.append(
            {
                "xq": np.ascontiguousarray(q[b].astype(f32).T).astype(NPBF),
                "xk": np.ascontiguousarray(k[b].astype(f32).T).astype(NPBF),
                "xv": np.ascontiguousarray(v[b].astype(f32).T).astype(NPBF),
                "wq": wq_t,
                "wk": wk_t,
                "wv": wv_t,
                "wo": wo_t,
                "bq": bq_l,
                "bk": bk_l,
                "bvb": bvb,
                "bob": bob,
                "mt": np.ascontiguousarray(mask[b, 0].T.astype(f32)).astype(NPBF),
            }
        )
    return in_maps


def kernel(q, k, v, mask, WQ_w, WQ_b, WK_w, WK_b, WV_w, WV_b, WO_w, WO_b):
    global _NC_CACHE, LAST_RESULTS
    if _NC_CACHE is None:
        _NC_CACHE = build_nc()
    nc = _NC_CACHE
    in_maps = prep_inputs(
        q, k, v, mask, WQ_w, WQ_b, WK_w, WK_b, WV_w, WV_b, WO_w, WO_b
    )
    res = run_bass_kernel_spmd(nc, in_maps, core_ids=list(range(B)))
    LAST_RESULTS = res
    return np.stack([res.results[b]["out"] for b in range(B)], axis=0).astype(
        np.float32
    )
